# revision 3
# baseline (speedup 1.0000x reference)
"""2-layer GCN on 8 Trainium2 NeuronCores.

Math (dense formulation):
    A~ = scatter_ones(edge_index) + I          (entries in {0,1,2}, exact in bf16)
    d  = clip(A~.sum(1), 1)^-1/2
    agg(H) = (d ⊙_row (A~ @ (d ⊙_row H)))      ("normalized aggregation")
    h   = relu(agg(x) @ W1 + b1)
    out = agg(h) @ W2 + b2

Sharding: rows of A~ (= output nodes) are split across 8 cores. Each core gets
A~.T[:, rows_i] in a partition-major layout ([128, n_k*rows] with k-chunk c at
column block c) and computes its row-slice of both aggregations on the tensor
engine (contraction over nodes on the partition axis, so the aggregation
output lands feature-major = exactly the lhsT layout the following
weight-matmul needs). The inner d-scaling is folded into x on the host; the
outer d-scaling + bias + relu run on DVE/ACT per 128-row block. Between the
layers the scaled hidden features hs = d ⊙ relu(...) are AllGathered (bf16,
1MB per rank) so every core holds all nodes' features for the second
aggregation.

All DRAM layouts are partition-major so every DMA moves >=8KB per partition
line (the naive node-major layout caps DMA at ~1-2KB lines / ~200GB/s and
made DMA the co-bottleneck at 375us; this version targets PE-bound ~260us).
"""

import sys

if '/opt/trn_rl_repo' not in sys.path:
    sys.path.insert(0, '/opt/trn_rl_repo')

import numpy as np
import ml_dtypes

import concourse.bass as bass
import concourse.tile as tile
from concourse import bacc, mybir
from concourse.bass_utils import run_bass_kernel_spmd

N_CORES = 8
BF16 = mybir.dt.bfloat16
F32 = mybir.dt.float32

# filled by kernel() on each run; test.py reads exec_time_ns from here
LAST_RESULT = None

_NC_CACHE = {}


def build_gcn(n_nodes, in_f, hid, out_f):
    rows = n_nodes // N_CORES     # output rows per core
    n_k = n_nodes // 128          # contraction chunks (global)
    n_rb = rows // 128            # 128-row blocks per core
    rw = min(512, rows)           # row free-dim chunk for aggregation matmuls
    n_rh = rows // rw
    n_fi = in_f // 128
    n_fh = hid // 128
    KB = min(4, n_k)              # k-chunks per AT stream DMA
    n_g = n_k // KB
    XC = min(16, n_k)             # k-chunks per resident-x chunk
    n_xc = n_k // XC

    nc = bacc.Bacc(num_devices=N_CORES)

    at_ext = nc.declare_dram_parameter("at", [128, n_k * rows], BF16, isOutput=False)
    xs_ext = nc.declare_dram_parameter("xs", [128, n_k * in_f], BF16, isOutput=False)
    w1_ext = nc.declare_dram_parameter("w1", [in_f, hid], BF16, isOutput=False)
    w2_ext = nc.declare_dram_parameter("w2", [hid, out_f], BF16, isOutput=False)
    b1_ext = nc.declare_dram_parameter("b1bc", [128, hid], F32, isOutput=False)
    b2_ext = nc.declare_dram_parameter("b2bc", [128, out_f], F32, isOutput=False)
    dr_ext = nc.declare_dram_parameter("dr8", [128, n_rb], F32, isOutput=False)
    out_ext = nc.declare_dram_parameter("out", [rows, out_f], F32, isOutput=True)

    # hs in partition-major layout: [p, rb*hid + f] = hs[rb*128+p, f]
    hs_local = nc.dram_tensor("hs_local", [128, n_rb * hid], BF16)
    hs_all = nc.dram_tensor(
        "hs_all", [N_CORES * 128, n_rb * hid], BF16, addr_space="Shared"
    )

    with tile.TileContext(nc) as tc:
        with (
            tc.tile_pool(name="const", bufs=1) as const_pool,
            tc.tile_pool(name="stream", bufs=3) as stream,
            tc.tile_pool(name="xsrc", bufs=1) as xsrc,
            tc.tile_pool(name="feat", bufs=max(n_fi, n_fh)) as feat,
            tc.tile_pool(name="ep", bufs=2) as ep,
            tc.tile_pool(name="psum", bufs=8, space="PSUM") as psum,
        ):
            # resident constants
            w1t = []
            for fc in range(n_fi):
                t = const_pool.tile([128, hid], BF16, tag=f"w1_{fc}")
                nc.sync.dma_start(t[:], w1_ext[fc * 128:(fc + 1) * 128, :])
                w1t.append(t)
            w2t = []
            for fc in range(n_fh):
                t = const_pool.tile([128, out_f], BF16, tag=f"w2_{fc}")
                nc.sync.dma_start(t[:], w2_ext[fc * 128:(fc + 1) * 128, :])
                w2t.append(t)
            b1t = const_pool.tile([128, hid], F32, tag="b1")
            nc.sync.dma_start(b1t[:], b1_ext[:])
            b2t = const_pool.tile([128, out_f], F32, tag="b2")
            nc.sync.dma_start(b2t[:], b2_ext[:])
            drt = const_pool.tile([128, n_rb], F32, tag="dr")
            nc.sync.dma_start(drt[:], dr_ext[:])

            # resident xs (partition-major), chunked so matmuls start early
            xsr = []
            for c in range(n_xc):
                t = xsrc.tile([128, XC * in_f], BF16, tag=f"xsr_{c}")
                nc.sync.dma_start(
                    t[:], xs_ext[:, c * XC * in_f:(c + 1) * XC * in_f]
                )
                xsr.append(t)

            def xs_slice(k, f):
                c, kk = k // XC, k % XC
                return xsr[c][:, kk * in_f + f * 128: kk * in_f + (f + 1) * 128]

            hsg = []  # filled after the AllGather

            def hs_slice(k, f):
                i, kk = k // n_rb, k % n_rb
                return hsg[i][:, kk * hid + f * 128: kk * hid + (f + 1) * 128]

            def aggregate(src_slice, n_f, label):
                """P_T[f, r] = sum_n src[n, f] * A~[r, n], feature-major psum."""
                acc = [
                    psum.tile([128, rw], F32, tag="acc", name=f"acc_{label}_{i}")
                    for i in range(n_f * n_rh)
                ]
                for g in range(n_g):
                    atq = stream.tile([128, KB * rows], BF16, tag="atq",
                                      name=f"atq_{label}_{g}")
                    nc.sync.dma_start(
                        atq[:], at_ext[:, g * KB * rows:(g + 1) * KB * rows]
                    )
                    for kk in range(KB):
                        k = g * KB + kk
                        for f in range(n_f):
                            for rh in range(n_rh):
                                nc.tensor.matmul(
                                    acc[f * n_rh + rh][:],
                                    src_slice(k, f),
                                    atq[:, kk * rows + rh * rw:
                                        kk * rows + (rh + 1) * rw],
                                    start=(k == 0),
                                    stop=(k == n_k - 1),
                                )
                # drain feature-major accumulation to SBUF (cast bf16)
                ps = []
                for f in range(n_f):
                    t = feat.tile([128, rows], BF16, tag="ps", name=f"ps_{label}_{f}")
                    for rh in range(n_rh):
                        nc.vector.tensor_copy(
                            t[:, rh * rw:(rh + 1) * rw], acc[f * n_rh + rh][:]
                        )
                    ps.append(t)
                return ps

            # ---- layer 1 ----
            p1s = aggregate(xs_slice, n_fi, "agg1")
            for rb in range(n_rb):
                zp = psum.tile([128, hid], F32, tag="acc")
                for fc in range(n_fi):
                    nc.tensor.matmul(
                        zp[:],
                        p1s[fc][:, rb * 128:(rb + 1) * 128],
                        w1t[fc][:],
                        start=(fc == 0),
                        stop=(fc == n_fi - 1),
                    )
                v = ep.tile([128, hid], F32, tag="v1")
                nc.vector.tensor_scalar_mul(v[:], zp[:], drt[:, rb:rb + 1])
                v2 = ep.tile([128, hid], F32, tag="v2")
                nc.vector.tensor_add(v2[:], v[:], b1t[:])
                hst = ep.tile([128, hid], BF16, tag="hst")
                nc.scalar.activation(
                    hst[:], v2[:], mybir.ActivationFunctionType.Relu,
                    scale=drt[:, rb:rb + 1],
                )
                nc.sync.dma_start(hs_local[:, rb * hid:(rb + 1) * hid], hst[:])

            nc.gpsimd.collective_compute(
                "AllGather",
                mybir.AluOpType.bypass,
                replica_groups=[list(range(N_CORES))],
                ins=[hs_local[:]],
                outs=[hs_all[:]],
            )

            # gathered hidden features, one tile per source rank
            for i in range(N_CORES):
                t = xsrc.tile([128, n_rb * hid], BF16, tag=f"hsg_{i}")
                nc.sync.dma_start(t[:], hs_all[i * 128:(i + 1) * 128, :])
                hsg.append(t)

            # ---- layer 2 ----
            p2s = aggregate(hs_slice, n_fh, "agg2")
            for rb in range(n_rb):
                zp = psum.tile([128, out_f], F32, tag="acc")
                for fc in range(n_fh):
                    nc.tensor.matmul(
                        zp[:],
                        p2s[fc][:, rb * 128:(rb + 1) * 128],
                        w2t[fc][:],
                        start=(fc == 0),
                        stop=(fc == n_fh - 1),
                    )
                v = ep.tile([128, out_f], F32, tag="vo1")
                nc.vector.tensor_scalar_mul(v[:], zp[:], drt[:, rb:rb + 1])
                o = ep.tile([128, out_f], F32, tag="vo2")
                nc.vector.tensor_add(o[:], v[:], b2t[:])
                nc.sync.dma_start(out_ext[rb * 128:(rb + 1) * 128, :], o[:])

    nc.finalize()
    return nc


def _to_partition_major(a, n_k):
    """[n_k*128, F] row-major -> [128, n_k*F] with chunk c at columns c*F."""
    f = a.shape[1]
    return np.ascontiguousarray(
        a.reshape(n_k, 128, f).transpose(1, 0, 2).reshape(128, n_k * f)
    )


def prep_inputs(x, edge_index, W1, b1, W2, b2):
    """Host-side prep: dense normalized adjacency + per-core shards."""
    x = np.asarray(x, dtype=np.float32)
    edge_index = np.asarray(edge_index)
    W1 = np.asarray(W1, dtype=np.float32)
    b1 = np.asarray(b1, dtype=np.float32)
    W2 = np.asarray(W2, dtype=np.float32)
    b2 = np.asarray(b2, dtype=np.float32)

    n = x.shape[0]
    rows = n // N_CORES
    n_rb = rows // 128
    n_k = n // 128

    adj = np.zeros((n, n), dtype=np.float32)
    adj[edge_index[0], edge_index[1]] = 1.0
    idx = np.arange(n)
    adj[idx, idx] += 1.0
    deg = np.maximum(adj.sum(axis=1), 1.0)
    dinv = (deg ** -0.5).astype(np.float32)

    xs = _to_partition_major((x * dinv[:, None]).astype(ml_dtypes.bfloat16), n_k)
    w1b = W1.astype(ml_dtypes.bfloat16)
    w2b = W2.astype(ml_dtypes.bfloat16)
    b1bc = np.ascontiguousarray(np.broadcast_to(b1, (128, b1.shape[0]))).astype(np.float32)
    b2bc = np.ascontiguousarray(np.broadcast_to(b2, (128, b2.shape[0]))).astype(np.float32)

    in_maps = []
    for i in range(N_CORES):
        sl = slice(i * rows, (i + 1) * rows)
        ati = np.ascontiguousarray(adj[sl, :].T).astype(ml_dtypes.bfloat16)
        in_maps.append({
            "at": _to_partition_major(ati, n_k),
            "xs": xs,
            "w1": w1b,
            "w2": w2b,
            "b1bc": b1bc,
            "b2bc": b2bc,
            "dr8": np.ascontiguousarray(dinv[sl].reshape(n_rb, 128).T),
        })
    return in_maps


def kernel(x, edge_index, W1, b1, W2, b2):
    global LAST_RESULT
    x = np.asarray(x)
    n, in_f = x.shape
    hid = np.asarray(W1).shape[1]
    out_f = np.asarray(W2).shape[1]

    key = (n, in_f, hid, out_f)
    if key not in _NC_CACHE:
        _NC_CACHE[key] = build_gcn(n, in_f, hid, out_f)
    nc = _NC_CACHE[key]

    in_maps = prep_inputs(x, edge_index, W1, b1, W2, b2)
    res = run_bass_kernel_spmd(nc, in_maps, core_ids=list(range(N_CORES)))
    LAST_RESULT = res
    return np.concatenate([res.results[i]["out"] for i in range(N_CORES)], axis=0)
